# revision 50
# baseline (speedup 1.0000x reference)
"""Trainium2 Bass kernel for nn_KairosModel (2-layer TransformerConv GNN + LinkPredictor).

Self-contained: takes full (unsharded) inputs, returns the full [E, 2] output.

v2 strategy (edge-parallel, dst-partitioned, fp16 compute / fp32 accumulate):
  * Host relabels nodes into W windows (<=128 nodes, <=S edges each) via a
    degree-sorted snake packing. Node slots are GROUP-major (G groups per
    core) so the two AllGathers split into G chunks that overlap compute.
    Segment-softmax stats are core-local; segment sums are one-hot matmuls.
  * rel_enc = cos(rel_t*w) and e2 = edge_attr @ c2_ew are computed ON THE
    HOST (the PE's fp32 multiply is 1 ulp off the CPU's, which at
    |arg|~3e6 is a 0.25 rad phase error -> was the dominant error source).
  * All matmuls run in fp16 (1 cyc/row, fp32 PSUM accumulate). The layer-1
    softmax exp is shifted by -4 so fp16 w = v*exp stays finite; the
    layer-2 scatter uses bf16 (fp32-range exponent, unshifted exp).
  * Only activation functions from one table set (Exp/Relu/Identity/Tanh)
    are used -> no act-table reloads.
"""
import sys

import numpy as np

if "/opt/trn_rl_repo" not in sys.path:
    sys.path.insert(0, "/opt/trn_rl_repo")

# ---------------- problem constants (hardcoded per contract) ----------------
N_FULL, E_FULL, D = 50000, 150000, 100
C = 8           # cores
NS = 128        # node slots per window
TS = 128        # edge slots per tile
G = 1           # collective chunks (groups of windows)
EXP_SHIFT = -4.0  # layer-1 softmax exp bias (cancels in the ratio)


# ---------------------------- host preprocessing ----------------------------

def preprocess(inputs, W, S):
    """Relabel nodes/edges into the padded window-slot space. Pure numpy.
    Node slots are group-major: slot(win) = (g*C*WPG + c*WPG + k)*NS with
    c = win // WPC, (g, k) = divmod(win % WPC, WPG)."""
    src = np.asarray(inputs["edge_index"][0]).astype(np.int64)
    dst = np.asarray(inputs["edge_index"][1]).astype(np.int64)
    x = np.asarray(inputs["x"], dtype=np.float32)
    N, E = x.shape[0], src.shape[0]
    WPC = W // C
    WPG = WPC // G
    NPAD = W * NS
    EPC = WPC * S

    deg = np.bincount(dst, minlength=N)
    order = np.argsort(-deg, kind="stable")
    k = np.arange(N) % (2 * W)
    win_of_sorted = np.where(k < W, k, 2 * W - 1 - k)
    win_of_node = np.empty(N, np.int64)
    win_of_node[order] = win_of_sorted

    wc = np.arange(W) // WPC
    wl = np.arange(W) % WPC
    wg, wk = wl // WPG, wl % WPG
    slot_base = (wg * C * WPG + wc * WPG + wk) * NS   # [W]

    by_win = np.argsort(win_of_node, kind="stable")
    wcnt = np.bincount(win_of_node, minlength=W)
    wstart = np.concatenate([[0], np.cumsum(wcnt)])[:W]
    pos = np.arange(N) - wstart[win_of_node[by_win]]
    gslot = np.empty(N, np.int64)
    gslot[by_win] = slot_base[win_of_node[by_win]] + pos
    assert wcnt.max() <= NS, wcnt.max()

    ewin = win_of_node[dst]
    ecnt = np.bincount(ewin, minlength=W)
    assert ecnt.max() <= S, f"bin overflow: {ecnt.max()} > {S}"
    eby = np.argsort(ewin, kind="stable")
    estart = np.concatenate([[0], np.cumsum(ecnt)])[:W]
    epos = np.arange(E) - estart[ewin[eby]]
    eslot = np.empty(E, np.int64)
    eslot[eby] = ewin[eby] * S + epos

    rel_t = (np.asarray(inputs["last_update"]).astype(np.int64)[src]
             - np.asarray(inputs["t"]).astype(np.int64)).astype(np.float32)

    # ---- host rel_enc (bit-exact vs the CPU reference) and e2 ----
    tw = np.asarray(inputs["time_w"], np.float32).reshape(1, D)
    tb = np.asarray(inputs["time_b"], np.float32).reshape(1, D)
    re = np.cos(rel_t[:, None] * tw + tb)                        # [E, D] f32
    c2ew = np.asarray(inputs["c2_ew"], np.float32)               # [200, D]
    msg = np.asarray(inputs["msg"], np.float32)
    e2 = re @ c2ew[:D] + msg @ c2ew[D:]                          # [E, D] f32

    x16 = x.astype(np.float16)
    xg16 = np.zeros((NPAD, D), np.float16)
    xg16[gslot] = x16

    psrc_g = np.zeros(W * S, np.int32)
    pedge_g = np.zeros(W * S, np.int32)
    dstloc_g = -np.ones(W * S, np.float32)
    reT_g = np.zeros((D, W * S), np.float16)
    e2_g = np.zeros((W * S, D), np.float32)
    psrc_g[eslot] = gslot[src].astype(np.int32)
    pedge_g[eslot] = np.arange(E, dtype=np.int32)
    dstloc_g[eslot] = (gslot[dst] % NS).astype(np.float32)
    reT_g[:, eslot] = re.T.astype(np.float16)
    e2_g[eslot] = e2

    T_TOT = WPC * (S // TS)
    per_core = []
    for c in range(C):
        sl = slice(c * EPC, (c + 1) * EPC)
        def tilize(a):
            return np.ascontiguousarray(a[sl].reshape(T_TOT, TS).T)
        xp = np.empty((WPC * NS, D), np.float16)
        for wl_ in range(WPC):
            g_, k_ = divmod(wl_, WPG)
            b = (g_ * C * WPG + c * WPG + k_) * NS
            xp[wl_ * NS:(wl_ + 1) * NS] = xg16[b:b + NS]
        per_core.append(dict(
            xp=np.ascontiguousarray(xp),
            psrcT=tilize(psrc_g),
            pedgeT=tilize(pedge_g),
            dstT=tilize(dstloc_g),
            reT=np.ascontiguousarray(reT_g[:, sl]),
            e2=np.ascontiguousarray(e2_g[sl]),
        ))
    return xg16, per_core, eslot


def fold_weights(inputs):
    f = {k: np.asarray(v, dtype=np.float32) if np.asarray(v).dtype.kind == "f"
         else np.asarray(v) for k, v in inputs.items()}
    out = {}
    aug16 = lambda w, b: np.ascontiguousarray(
        np.concatenate([w, b[None]], 0)).astype(np.float16)
    out["kw_aug"] = aug16(f["c1_kw"], f["c1_kb"])                    # [101,800]
    out["dvw_aug"] = aug16(f["c1_vw"] - f["c1_kw"], f["c1_vb"] - f["c1_kb"])
    out["qw_aug"] = aug16(f["c1_qw"], f["c1_qb"])
    out["sw_aug"] = aug16(f["c1_sw"], f["c1_sb"])
    out["ew16"] = np.ascontiguousarray(f["c1_ew"]).astype(np.float16)  # [200,800]
    w2 = np.concatenate([f["c2_qw"], f["c2_kw"], f["c2_vw"], f["c2_sw"]], 1)
    b2 = np.concatenate([f["c2_qb"], f["c2_kb"], f["c2_vb"], f["c2_sb"]])
    out["w2cat"] = aug16(w2, b2)                                     # [801,400]
    out["A_s"] = (f["lp_src_w"] @ f["lp1_w"][:200]).astype(np.float16)
    out["A_d"] = (f["lp_dst_w"] @ f["lp1_w"][200:]).astype(np.float16)
    out["b1p"] = np.ascontiguousarray(
        f["lp1_b"] + f["lp_src_b"] @ f["lp1_w"][:200]
        + f["lp_dst_b"] @ f["lp1_w"][200:]).reshape(-1, 1)           # [800,1] f32
    out["lp2_w"] = np.ascontiguousarray(f["lp2_w"]).astype(np.float16)
    out["lp2_b"] = f["lp2_b"].reshape(-1, 1)
    out["lp3_w"] = np.ascontiguousarray(f["lp3_w"]).astype(np.float16)
    out["lp3_b"] = f["lp3_b"].reshape(-1, 1)
    out["lp4_w"] = np.ascontiguousarray(f["lp4_w"]).astype(np.float16)
    out["lp4_b"] = f["lp4_b"].reshape(-1, 1)
    return out


# ------------------------------ device program ------------------------------

def _chunks(total, step=128):
    return [(s, min(step, total - s)) for s in range(0, total, step)]


def build_program(W, S, NPAD, E_msg):
    """Build the SPMD Bass/Tile program (identical for all 8 cores)."""
    from contextlib import ExitStack
    import concourse.bass as bass
    import concourse.mybir as mybir
    import concourse.tile as tile
    import concourse.bacc as bacc

    dt = mybir.dt
    AF = mybir.ActivationFunctionType
    OP = mybir.AluOpType
    AX = mybir.AxisListType

    WPC = W // C
    WPG = WPC // G
    TPW = S // TS
    NPC = WPC * NS
    EPC = WPC * S
    T_TOT = WPC * TPW

    nc = bacc.Bacc("TRN2", target_bir_lowering=False, debug=False,
                   num_devices=C)

    f32 = dt.float32
    f16 = dt.float16
    bf16 = dt.bfloat16

    # ---- DRAM I/O ----
    def di(n, sh, d=dt.float32):
        return nc.dram_tensor(n, sh, d, kind="ExternalInput").ap()
    xg_d = di("xg16", [NPAD, D], f16)
    xp_d = di("xp", [NPC, D], f16)
    msg_d = di("msg16", [E_msg, D], f16)
    reT_d = di("reT", [D, EPC], f16)
    e2_d = di("e2", [EPC, D])
    dstT_d = di("dstT", [TS, T_TOT])
    psrcT_d = di("psrcT", [TS, T_TOT], dt.int32)
    pedgeT_d = di("pedgeT", [TS, T_TOT], dt.int32)
    kw_d = di("kw_aug", [101, 800], f16)
    dvw_d = di("dvw_aug", [101, 800], f16)
    qw_d = di("qw_aug", [101, 800], f16)
    sw_d = di("sw_aug", [101, 800], f16)
    ew_d = di("ew16", [200, 800], f16)
    w2_d = di("w2cat", [801, 400], f16)
    As_d = di("A_s", [D, 800], f16)
    Ad_d = di("A_d", [D, 800], f16)
    b1p_d = di("b1p", [800, 1])
    lp2w_d = di("lp2_w", [800, 200], f16)
    lp2b_d = di("lp2_b", [200, 1])
    lp3w_d = di("lp3_w", [200, 50], f16)
    lp3b_d = di("lp3_b", [50, 1])
    lp4w_d = di("lp4_w", [50, 2], f16)
    lp4b_d = di("lp4_b", [2, 1])

    outT_d = nc.dram_tensor("outT", [2, EPC], dt.float32, kind="ExternalOutput").ap()

    # ---- internal DRAM ----
    kvc_g = [nc.dram_tensor(f"kv_c{g}", [WPG * NS, 200], f16).ap()
             for g in range(G)]
    kvt_d = nc.dram_tensor("kv_table", [NPAD, 200], f16,
                           addr_space="Shared").ap()
    h2c_g = [nc.dram_tensor(f"h2_c{g}", [WPG * NS, D], f16).ap()
             for g in range(G)]
    h2t_d = nc.dram_tensor("h2_table", [NPAD, D], f16,
                           addr_space="Shared").ap()
    s2_d = nc.dram_tensor("s2_spill", [NPC, D], dt.float32).ap()

    RG = [list(range(C))]

    with tile.TileContext(nc) as tc, ExitStack() as ctx:
        const = ctx.enter_context(tc.tile_pool(name="const", bufs=1))
        resid = ctx.enter_context(tc.tile_pool(name="resid", bufs=1))
        pwin = ctx.enter_context(tc.tile_pool(name="pwin", bufs=3))
        pt = ctx.enter_context(tc.tile_pool(name="pt", bufs=4))
        pbig = ctx.enter_context(tc.tile_pool(name="pbig", bufs=3))
        pe1 = ctx.enter_context(tc.tile_pool(name="pe1", bufs=2))
        ps_num = ctx.enter_context(tc.tile_pool(name="ps_num", bufs=2, space="PSUM"))
        ps_kj = ctx.enter_context(tc.tile_pool(name="ps_kj", bufs=1, space="PSUM"))
        ps_qv = ctx.enter_context(tc.tile_pool(name="ps_qv", bufs=1, space="PSUM"))

        # ---- constants ----
        bias_zero = const.tile([128, 1], f32)
        nc.vector.memset(bias_zero[:], 0.0)
        bias_shift = const.tile([128, 1], f32)
        nc.vector.memset(bias_shift[:], EXP_SHIFT)
        iota_i = const.tile([128, 128], dt.int32)
        nc.gpsimd.iota(iota_i[:], pattern=[[1, 128]], base=0, channel_multiplier=0)
        iota_f = const.tile([128, 128], f32)
        nc.vector.tensor_copy(iota_f[:], iota_i[:])
        iota_h = const.tile([128, 128], f16)
        nc.scalar.activation(iota_h[:], iota_f[:], AF.Identity,
                             bias=bias_zero[:], scale=1.0)
        ones_f = const.tile([1, 128], f32)
        nc.vector.memset(ones_f[:], 1.0)
        ones_h = const.tile([1, 128], f16)
        nc.vector.tensor_copy(ones_h[:], ones_f[:])

        def load_const(name, ap, shape, dtype=f32):
            t = const.tile(list(shape), dtype, name=name)
            nc.sync.dma_start(out=t[:], in_=ap)
            return t

        kw_s = load_const("kw_s", kw_d[:, :], [101, 800], f16)
        dvw_s = load_const("dvw_s", dvw_d[:, :], [101, 800], f16)
        qw_s = load_const("qw_s", qw_d[:, :], [101, 800], f16)
        sw_s = load_const("sw_s", sw_d[:, :], [101, 800], f16)
        ew0_s = load_const("ew0_s", ew_d[0:100, :], [100, 800], f16)
        ew1_s = load_const("ew1_s", ew_d[100:200, :], [100, 800], f16)
        As_s = load_const("As_s", As_d[:, :], [D, 800], f16)
        Ad_s = load_const("Ad_s", Ad_d[:, :], [D, 800], f16)
        w2_ch = []
        for c_, (s_, n_) in enumerate(_chunks(800)):
            t = const.tile([n_, 400], f16, name=f"w2ch{c_}")
            nc.sync.dma_start(out=t[:], in_=w2_d[s_:s_ + n_, :])
            w2_ch.append((t, n_))
        b2_s = load_const("b2_s", w2_d[800:801, :], [1, 400], f16)
        lp2_ch = []
        for c_, (s_, n_) in enumerate(_chunks(800)):
            t = const.tile([n_, 200], f16, name=f"lp2ch{c_}")
            nc.sync.dma_start(out=t[:], in_=lp2w_d[s_:s_ + n_, :])
            lp2_ch.append((t, n_))
        lp3w_s = const.tile([128, 100], f16)  # chunk k at cols [50k:50k+50]
        nc.sync.dma_start(out=lp3w_s[0:128, 0:50], in_=lp3w_d[0:128, :])
        nc.sync.dma_start(out=lp3w_s[0:72, 50:100], in_=lp3w_d[128:200, :])
        lp4w_s = load_const("lp4w_s", lp4w_d[:, :], [50, 2], f16)
        lp2b_s = const.tile([128, 2], f32)
        nc.sync.dma_start(out=lp2b_s[0:128, 0:1], in_=lp2b_d[0:128, :])
        nc.sync.dma_start(out=lp2b_s[0:72, 1:2], in_=lp2b_d[128:200, :])
        lp3b_s = load_const("lp3b_s", lp3b_d[:, :], [50, 1])
        lp4b_s = load_const("lp4b_s", lp4b_d[:, :], [2, 1])
        b1p_s = const.tile([128, 7], f32)
        for c_, (s_, n_) in enumerate(_chunks(800)):
            nc.sync.dma_start(out=b1p_s[0:n_, c_:c_ + 1], in_=b1p_d[s_:s_ + n_, :])

        # ---- per-core index tables ----
        dstT_s = const.tile([TS, T_TOT], f32)
        nc.sync.dma_start(out=dstT_s[:], in_=dstT_d[:, :])
        psrcT_s = const.tile([TS, T_TOT], dt.int32)
        nc.sync.dma_start(out=psrcT_s[:], in_=psrcT_d[:, :])
        pedgeT_s = const.tile([TS, T_TOT], dt.int32)
        nc.sync.dma_start(out=pedgeT_s[:], in_=pedgeT_d[:, :])

        # ---- residents ----
        q2_all = resid.tile([NS, WPC * D], f16)
        h2_all = resid.tile([NS, WPC * D], f16)

        def build_S16(t):
            """fp16 one-hot S [128e,128n] and its fp16 transpose St."""
            S_sb = pt.tile([TS, NS], f16, tag="S_sb")
            nc.vector.tensor_scalar(out=S_sb[:], in0=iota_h[:],
                                    scalar1=dstT_s[:, t:t + 1], scalar2=None,
                                    op0=OP.is_equal)
            St_sb = pt.tile([NS, TS], f16, tag="St_sb")
            nc.sync.dma_start_transpose(out=St_sb[:], in_=S_sb[:])
            return S_sb, St_sb

        # =================== phase A (layer 1 + projections) ===========
        for w in range(WPC):
            g_, k_ = divmod(w, WPG)
            xp_blk = pwin.tile([NS, 128], f16, tag="xp_blk")
            nc.sync.dma_start(out=xp_blk[:, 0:D], in_=xp_d[w * NS:(w + 1) * NS, :])
            nc.vector.memset(xp_blk[:, D:D + 1], 1.0)
            xpT_pad = pwin.tile([128, NS], f16, tag="xpT_pad")
            nc.sync.dma_start_transpose(out=xpT_pad[:], in_=xp_blk[:])
            xpT_aug = xpT_pad[0:D + 1, :]

            q_ps_a = ps_qv.tile([NS, 400], f32, tag="qv_a")
            q_ps_b = ps_qv.tile([NS, 400], f32, tag="qv_b")
            nc.tensor.matmul(out=q_ps_a[:], lhsT=xpT_aug, rhs=qw_s[:, 0:400],
                             start=True, stop=True)
            nc.tensor.matmul(out=q_ps_b[:], lhsT=xpT_aug, rhs=qw_s[:, 400:800],
                             start=True, stop=True)
            q_blk = pwin.tile([NS, 800], f16, tag="q_blk")
            nc.scalar.copy(out=q_blk[:, 0:400], in_=q_ps_a[:])
            nc.scalar.copy(out=q_blk[:, 400:800], in_=q_ps_b[:])

            num_a = ps_num.tile([NS, 400], f32, tag="num_a")
            num_b = ps_num.tile([NS, 408], f32, tag="num_b")

            for j in range(TPW):
                t = w * TPW + j
                S_sb, St_sb = build_S16(t)

                # q_dst expansion
                qd_a = ps_qv.tile([TS, 400], f32, tag="qv_a")
                qd_b = ps_qv.tile([TS, 400], f32, tag="qv_b")
                nc.tensor.matmul(out=qd_a[:], lhsT=St_sb[:], rhs=q_blk[:, 0:400],
                                 start=True, stop=True)
                nc.tensor.matmul(out=qd_b[:], lhsT=St_sb[:], rhs=q_blk[:, 400:800],
                                 start=True, stop=True)

                # gather x[src] (fp16), append ones col, transpose
                xs = pt.tile([TS, 128], f16, tag="xs")
                nc.gpsimd.indirect_dma_start(
                    out=xs[:, 0:D], out_offset=None, in_=xg_d[:, :],
                    in_offset=bass.IndirectOffsetOnAxis(ap=psrcT_s[:, t:t + 1], axis=0))
                nc.vector.memset(xs[:, D:D + 1], 1.0)
                xsT_pad = pt.tile([128, TS], f16, tag="xsT_pad")
                nc.sync.dma_start_transpose(out=xsT_pad[:], in_=xs[:])
                xsT_aug = xsT_pad[0:D + 1, :]

                # rel_enc from host (fp16); msg gather + transpose
                re_sb = pt.tile([D, TS], f16, tag="re_sb")
                nc.sync.dma_start(out=re_sb[:], in_=reT_d[:, t * TS:(t + 1) * TS])
                mq = pt.tile([TS, 128], f16, tag="mq")
                nc.gpsimd.indirect_dma_start(
                    out=mq[:, 0:D], out_offset=None, in_=msg_d[:, :],
                    in_offset=bass.IndirectOffsetOnAxis(ap=pedgeT_s[:, t:t + 1], axis=0))
                mg_pad = pt.tile([128, TS], f16, tag="mg_pad")
                nc.sync.dma_start_transpose(out=mg_pad[:], in_=mq[:])
                mg_sb = mg_pad[0:D, :]

                # kj = x@kw_aug + re@ew0 + mg@ew1   [128e, 800]
                kj_a = ps_kj.tile([TS, 400], f32, tag="kj_a")
                kj_b = ps_kj.tile([TS, 400], f32, tag="kj_b")
                for half, kp in ((0, kj_a), (1, kj_b)):
                    sl = slice(half * 400, half * 400 + 400)
                    nc.tensor.matmul(out=kp[:], lhsT=xsT_aug, rhs=kw_s[:, sl],
                                     start=True, stop=False)
                    nc.tensor.matmul(out=kp[:], lhsT=re_sb[:], rhs=ew0_s[:, sl],
                                     start=False, stop=False)
                    nc.tensor.matmul(out=kp[:], lhsT=mg_sb, rhs=ew1_s[:, sl],
                                     start=False, stop=True)

                # evict kj (fp16)
                kj_sb = pbig.tile([TS, 800], f16, tag="kj_sb")
                nc.scalar.copy(out=kj_sb[:, 0:400], in_=kj_a[:])
                nc.scalar.copy(out=kj_sb[:, 400:800], in_=kj_b[:])

                # alpha = sum_d q_dst*kj (per head); ex = exp(alpha/10 - 4)
                prod = pbig.tile([TS, 800], f32, tag="prod")
                nc.vector.tensor_tensor(out=prod[:, 0:400], in0=kj_sb[:, 0:400],
                                        in1=qd_a[:], op=OP.mult)
                nc.vector.tensor_tensor(out=prod[:, 400:800], in0=kj_sb[:, 400:800],
                                        in1=qd_b[:], op=OP.mult)
                alpha = pt.tile([TS, 8], f32, tag="alpha")
                nc.vector.reduce_sum(out=alpha[:],
                                     in_=prod[:].rearrange("p (h d) -> p h d", d=D),
                                     axis=AX.X)

                # w = kj + x@dvw: accumulate onto the kj PSUM after the evict
                for half, kp in ((0, kj_a), (1, kj_b)):
                    sl = slice(half * 400, half * 400 + 400)
                    nc.tensor.matmul(out=kp[:], lhsT=xsT_aug, rhs=dvw_s[:, sl],
                                     start=False, stop=True)
                w_sb = pbig.tile([TS, 808], f16, tag="w_sb")
                nc.scalar.copy(out=w_sb[:, 0:400], in_=kj_a[:])
                nc.scalar.copy(out=w_sb[:, 400:800], in_=kj_b[:])
                ex = pt.tile([TS, 8], f32, tag="ex")
                nc.scalar.activation(ex[:], alpha[:], AF.Exp,
                                     bias=bias_shift[0:TS, :], scale=0.1)
                for h in range(8):
                    nc.gpsimd.tensor_scalar_mul(
                        out=w_sb[:, h * D:(h + 1) * D],
                        in0=w_sb[:, h * D:(h + 1) * D], scalar1=ex[:, h:h + 1])
                nc.gpsimd.tensor_copy(out=w_sb[:, 800:808], in_=ex[:])

                # scatter-add into per-window numerator/denominator
                nc.tensor.matmul(out=num_a[:], lhsT=S_sb[:], rhs=w_sb[:, 0:400],
                                 start=(j == 0), stop=(j == TPW - 1))
                nc.tensor.matmul(out=num_b[:], lhsT=S_sb[:], rhs=w_sb[:, 400:808],
                                 start=(j == 0), stop=(j == TPW - 1))

            # ---- window eviction: h1 = relu(num/den + x@sw_aug) ----
            den = pwin.tile([NS, 8], f32, tag="den")
            nc.vector.tensor_scalar_add(out=den[:], in0=num_b[:, 400:408],
                                        scalar1=1e-16)
            rcp = pwin.tile([NS, 8], f32, tag="rcp")
            nc.vector.reciprocal(out=rcp[:], in_=den[:])
            agg = pwin.tile([NS, 800], f32, tag="agg")
            for h in range(8):
                src_ps = num_a if h < 4 else num_b
                off = h * D if h < 4 else (h - 4) * D
                if h % 2 == 0:
                    nc.scalar.activation(agg[:, h * D:(h + 1) * D],
                                         src_ps[:, off:off + D], AF.Identity,
                                         bias=bias_zero[0:NS, :],
                                         scale=rcp[:, h:h + 1])
                else:
                    nc.vector.tensor_scalar_mul(out=agg[:, h * D:(h + 1) * D],
                                                in0=src_ps[:, off:off + D],
                                                scalar1=rcp[:, h:h + 1])
            skip_a = ps_kj.tile([NS, 400], f32, tag="kj_a")
            skip_b = ps_kj.tile([NS, 400], f32, tag="kj_b")
            nc.tensor.matmul(out=skip_a[:], lhsT=xpT_aug, rhs=sw_s[:, 0:400],
                             start=True, stop=True)
            nc.tensor.matmul(out=skip_b[:], lhsT=xpT_aug, rhs=sw_s[:, 400:800],
                             start=True, stop=True)
            h1 = pwin.tile([NS, 896], f16, tag="h1")
            nc.vector.tensor_tensor(out=h1[:, 0:400], in0=agg[:, 0:400],
                                    in1=skip_a[:], op=OP.add)
            nc.vector.tensor_tensor(out=h1[:, 400:800], in0=agg[:, 400:800],
                                    in1=skip_b[:], op=OP.add)
            nc.vector.tensor_scalar_max(out=h1[:, 0:800], in0=h1[:, 0:800],
                                        scalar1=0.0)

            # h1^T chunks -> y2 = [q2|k2|v2|s2]
            h1T = pwin.tile([128, 7 * 128], f16, tag="h1T")
            for c_ in range(7):
                nc.sync.dma_start_transpose(
                    out=h1T[:, c_ * 128:(c_ + 1) * 128],
                    in_=h1[:, c_ * 128:(c_ + 1) * 128])
            y2 = ps_qv.tile([NS, 400], f32, tag="qv_a")
            for c_, (w2t, n_) in enumerate(w2_ch):
                nc.tensor.matmul(out=y2[:], lhsT=h1T[0:n_, c_ * 128:c_ * 128 + NS],
                                 rhs=w2t[:], start=(c_ == 0), stop=False)
            nc.tensor.matmul(out=y2[:], lhsT=ones_h[:], rhs=b2_s[:],
                             start=False, stop=True)
            nc.scalar.copy(out=q2_all[:, w * D:(w + 1) * D], in_=y2[:, 0:D])
            s2_sb = pwin.tile([NS, D], f32, tag="s2_sb")
            nc.scalar.copy(out=s2_sb[:], in_=y2[:, 300:400])
            nc.sync.dma_start(out=s2_d[w * NS:(w + 1) * NS, :], in_=s2_sb[:])
            kv_sb = pwin.tile([NS, 200], f16, tag="kv_sb")
            nc.scalar.copy(out=kv_sb[:], in_=y2[:, 100:300])
            nc.sync.dma_start(out=kvc_g[g_][k_ * NS:(k_ + 1) * NS, :], in_=kv_sb[:])

            if k_ == WPG - 1:
                lo = g_ * C * WPG * NS
                hi = (g_ + 1) * C * WPG * NS
                nc.gpsimd.collective_compute(
                    "AllGather", mybir.AluOpType.bypass, replica_groups=RG,
                    ins=[kvc_g[g_].opt()], outs=[kvt_d[lo:hi, :].opt()])

        # =================== phase C (layer 2) =============================
        for w in range(WPC):
            g_, k_ = divmod(w, WPG)
            num2 = ps_num.tile([NS, 104], f32, tag="num_a")
            for j in range(TPW):
                t = w * TPW + j
                S_sb, St_sb = build_S16(t)
                q2d_ps = ps_qv.tile([TS, D], f32, tag="qv_a")
                nc.tensor.matmul(out=q2d_ps[:], lhsT=St_sb[:],
                                 rhs=q2_all[:, w * D:(w + 1) * D],
                                 start=True, stop=True)
                e2_s = pt.tile([TS, D], f32, tag="e2_s")
                nc.sync.dma_start(out=e2_s[:], in_=e2_d[t * TS:(t + 1) * TS, :])
                kvg = pt.tile([TS, 200], f16, tag="kvg")
                nc.gpsimd.indirect_dma_start(
                    out=kvg[:], out_offset=None, in_=kvt_d[:, :],
                    in_offset=bass.IndirectOffsetOnAxis(ap=psrcT_s[:, t:t + 1], axis=0))
                kj2 = pt.tile([TS, D], f32, tag="kj2")
                nc.gpsimd.tensor_tensor(out=kj2[:], in0=kvg[:, 0:D], in1=e2_s[:],
                                        op=OP.add)
                scr2 = pt.tile([TS, D], f32, tag="scr2")
                nc.vector.tensor_tensor(out=scr2[:], in0=kj2[:], in1=q2d_ps[:],
                                        op=OP.mult)
                alpha2 = pt.tile([TS, 1], f32, tag="alpha2")
                nc.vector.reduce_sum(out=alpha2[:], in_=scr2[:], axis=AX.X)
                ex2 = pt.tile([TS, 1], f32, tag="ex2")
                nc.scalar.activation(ex2[:], alpha2[:], AF.Exp,
                                     bias=bias_zero[0:TS, :], scale=0.1)
                w2_sb = pt.tile([TS, 101], bf16, tag="w2_sb")
                nc.gpsimd.tensor_tensor(out=w2_sb[:, 0:D], in0=kvg[:, D:200],
                                        in1=e2_s[:], op=OP.add)
                nc.vector.tensor_scalar_mul(out=w2_sb[:, 0:D], in0=w2_sb[:, 0:D],
                                            scalar1=ex2[:])
                nc.vector.tensor_copy(out=w2_sb[:, D:101], in_=ex2[:])
                S_bf = pt.tile([TS, NS], bf16, tag="S_bf")
                nc.vector.tensor_scalar(out=S_bf[:], in0=iota_f[:],
                                        scalar1=dstT_s[:, t:t + 1], scalar2=None,
                                        op0=OP.is_equal)
                nc.tensor.matmul(out=num2[:, 0:101], lhsT=S_bf[:], rhs=w2_sb[:],
                                 start=(j == 0), stop=(j == TPW - 1))
            den2 = pwin.tile([NS, 1], f32, tag="den2")
            nc.vector.tensor_scalar_add(out=den2[:], in0=num2[:, D:D + 1],
                                        scalar1=1e-16)
            rcp2 = pwin.tile([NS, 1], f32, tag="rcp2")
            nc.vector.reciprocal(out=rcp2[:], in_=den2[:])
            s2_sb = pwin.tile([NS, D], f32, tag="s2_sb")
            nc.scalar.dma_start(out=s2_sb[:], in_=s2_d[w * NS:(w + 1) * NS, :])
            h2f = pwin.tile([NS, D], f32, tag="h2f")
            nc.vector.tensor_scalar_mul(out=h2f[:], in0=num2[:, 0:D], scalar1=rcp2[:])
            nc.vector.tensor_tensor(out=h2f[:], in0=h2f[:], in1=s2_sb[:], op=OP.add)
            nc.scalar.activation(h2f[:], h2f[:], AF.Relu,
                                 bias=bias_zero[0:NS, :], scale=1.0)
            nc.vector.tensor_copy(out=h2_all[:, w * D:(w + 1) * D], in_=h2f[:])
            nc.sync.dma_start(out=h2c_g[g_][k_ * NS:(k_ + 1) * NS, :],
                              in_=h2_all[:, w * D:(w + 1) * D])

            if k_ == WPG - 1:
                lo = g_ * C * WPG * NS
                hi = (g_ + 1) * C * WPG * NS
                nc.gpsimd.collective_compute(
                    "AllGather", mybir.AluOpType.bypass, replica_groups=RG,
                    ins=[h2c_g[g_].opt()], outs=[h2t_d[lo:hi, :].opt()])

        # =================== phase E (LinkPredictor) =======================
        SE = S  # whole window's edges at once, feature-major
        for w in range(WPC):
            St_w = pt.tile([NS, SE], f16, tag="St_w")
            hsT = pt.tile([128, SE], f16, tag="hsT")
            for j in range(TPW):
                t = w * TPW + j
                S_sb = pt.tile([TS, NS], f16, tag="S_sb")
                nc.vector.tensor_scalar(out=S_sb[:], in0=iota_h[:],
                                        scalar1=dstT_s[:, t:t + 1], scalar2=None,
                                        op0=OP.is_equal)
                nc.sync.dma_start_transpose(out=St_w[:, j * TS:(j + 1) * TS],
                                            in_=S_sb[:])
                hs = pt.tile([TS, 128], f16, tag="hs")
                nc.gpsimd.indirect_dma_start(
                    out=hs[:, 0:D], out_offset=None, in_=h2t_d[:, :],
                    in_offset=bass.IndirectOffsetOnAxis(ap=psrcT_s[:, t:t + 1], axis=0))
                nc.sync.dma_start_transpose(out=hsT[:, j * TS:(j + 1) * TS],
                                            in_=hs[:])
            hdT_ps = ps_qv.tile([D, SE], f32, tag="qv_a")
            nc.tensor.matmul(out=hdT_ps[:], lhsT=h2_all[:, w * D:(w + 1) * D],
                             rhs=St_w[:], start=True, stop=True)
            hdT = pt.tile([D, SE], f16, tag="hdT")
            nc.vector.tensor_copy(out=hdT[:], in_=hdT_ps[:])

            t1 = pe1.tile([128, 7 * SE], f16, tag="t1")
            for c_, (s_, n_) in enumerate(_chunks(800)):
                t1_ps = ps_num.tile([128, SE], f32,
                                    tag="num_a" if c_ % 2 == 0 else "num_b")
                nc.tensor.matmul(out=t1_ps[0:n_, :], lhsT=As_s[:, s_:s_ + n_],
                                 rhs=hsT[0:D, :], start=True, stop=False)
                nc.tensor.matmul(out=t1_ps[0:n_, :], lhsT=Ad_s[:, s_:s_ + n_],
                                 rhs=hdT[:], start=False, stop=True)
                nc.scalar.activation(t1[0:n_, c_ * SE:(c_ + 1) * SE], t1_ps[0:n_, :],
                                     AF.Tanh, bias=b1p_s[0:n_, c_:c_ + 1], scale=1.0)
            t2 = pe1.tile([128, 2 * SE], f16, tag="t2")
            for m, (ms, mn) in enumerate(_chunks(200)):
                t2_ps = ps_kj.tile([128, SE], f32, tag="kj_a" if m == 0 else "kj_b")
                for c_, (lp2t, n_) in enumerate(lp2_ch):
                    nc.tensor.matmul(out=t2_ps[0:mn, :],
                                     lhsT=lp2t[:, ms:ms + mn],
                                     rhs=t1[0:n_, c_ * SE:(c_ + 1) * SE],
                                     start=(c_ == 0), stop=(c_ == 6))
                nc.scalar.activation(t2[0:mn, m * SE:(m + 1) * SE], t2_ps[0:mn, :],
                                     AF.Tanh, bias=lp2b_s[0:mn, m:m + 1], scale=1.0)
            t3_ps = ps_qv.tile([50, SE], f32, tag="qv_b")
            nc.tensor.matmul(out=t3_ps[:], lhsT=lp3w_s[0:128, 0:50],
                             rhs=t2[0:128, 0:SE], start=True, stop=False)
            nc.tensor.matmul(out=t3_ps[:], lhsT=lp3w_s[0:72, 50:100],
                             rhs=t2[0:72, SE:2 * SE], start=False, stop=True)
            t3 = pwin.tile([50, SE], f16, tag="t3")
            nc.scalar.activation(t3[:], t3_ps[:], AF.Tanh, bias=lp3b_s[:], scale=1.0)
            out_ps = ps_num.tile([2, SE], f32, tag="num_b")
            nc.tensor.matmul(out=out_ps[:], lhsT=lp4w_s[:], rhs=t3[:],
                             start=True, stop=True)
            out_sb = pwin.tile([2, SE], f32, tag="out_sb")
            nc.vector.tensor_scalar(out=out_sb[:], in0=out_ps[:],
                                    scalar1=lp4b_s[:, 0:1], scalar2=None,
                                    op0=OP.add)
            nc.sync.dma_start(out=outT_d[:, w * S:(w + 1) * S], in_=out_sb[:])

    nc.compile()
    return nc


# ------------------------------- entry point -------------------------------

_CACHE = {}
_LAST_S = 384


def _get_program(W, S, NPAD, E_msg):
    key = (W, S, NPAD, E_msg)
    if key not in _CACHE:
        _CACHE[key] = build_program(W, S, NPAD, E_msg)
    return _CACHE[key]


def build_inmaps(inputs, W, S):
    xg16, per_core, eslot = preprocess(inputs, W, S)
    fw = fold_weights(inputs)
    msg16 = np.ascontiguousarray(np.asarray(inputs["msg"]).astype(np.float16))
    shared = dict(xg16=xg16, msg16=msg16, **fw)
    in_maps = []
    for c in range(C):
        pc = per_core[c]
        in_maps.append(dict(shared, xp=pc["xp"], reT=pc["reT"], e2=pc["e2"],
                            dstT=pc["dstT"], psrcT=pc["psrcT"],
                            pedgeT=pc["pedgeT"]))
    return in_maps, eslot


def kernel(**inputs):
    global _LAST_S
    W = 400
    for S_try in (384, 512, 640, 768):
        try:
            in_maps, eslot = build_inmaps(inputs, W, S_try)
            S = S_try
            break
        except AssertionError:
            continue
    else:
        raise RuntimeError("window packing failed")
    _LAST_S = S

    NPAD = W * NS
    nc = _get_program(W, S, NPAD, E_FULL)

    from concourse import bass_utils
    res = bass_utils.run_bass_kernel_spmd(nc, in_maps, core_ids=list(range(C)))

    outT = np.stack([res.results[c]["outT"] for c in range(C)])  # [C,2,EPC]
    flat = outT.transpose(0, 2, 1).reshape(W * S, 2)             # [W*S, 2]
    return np.ascontiguousarray(flat[eslot]).astype(np.float32)
